# revision 1
# baseline (speedup 1.0000x reference)
"""CapsuleLayer dynamic-routing kernel for one TRN2 chip (8 NeuronCores).

Self-contained graded entry point: kernel(x, W) -> [128, 32, 16] float32.
"""

import sys

sys.path.insert(0, "/opt/trn_rl_repo")

import numpy as np
import ml_dtypes

import concourse.bass as bass
import concourse.bacc as bacc
import concourse.mybir as mybir
from concourse.tile import TileContext

BF16 = mybir.dt.bfloat16
F32 = mybir.dt.float32
AX = mybir.AxisListType
ALU = mybir.AluOpType
ACTF = mybir.ActivationFunctionType

B, C, I, J, U = 128, 2048, 16, 32, 16
JU = J * U  # 512
EPS = 1e-8
N_ITERS = 3


def build_nc(c_loc=256, n_cores=8, debug_taps=False, n_iters=N_ITERS, do_ar=True, reps=1):
    NCH = c_loc // 4          # chunks of 4 capsules
    NSG = (NCH + 3) // 4      # supergroups of 4 chunks (16 capsules)
    nc = bacc.Bacc(None, target_bir_lowering=False, debug=True)
    xt_d = nc.declare_dram_parameter("xt", [128, NCH * 128], BF16, isOutput=False)
    wr_d = nc.declare_dram_parameter("wr", [128, NCH * 512], BF16, isOutput=False)
    out_d = nc.declare_dram_parameter("out", [128, JU], F32, isOutput=True)
    taps = {}
    if debug_taps:
        for nm, cols, dt in [("s0", JU, F32), ("v0", JU, F32), ("b1", c_loc * J, F32),
                             ("ct1", c_loc * J, BF16), ("s1", JU, F32),
                             ("b2", c_loc * J, F32), ("ct2", c_loc * J, BF16),
                             ("us0", 16 * 512, BF16)]:
            taps[nm] = nc.declare_dram_parameter("tap_" + nm, [128, cols], dt, isOutput=True)

    def tap(nm, ap):
        if debug_taps:
            nc.sync.dma_start(out=taps[nm][:, :], in_=ap)

    with TileContext(nc) as tc:
        with (
            tc.tile_pool(name="const", bufs=1) as cpool,
            tc.tile_pool(name="stage", bufs=2) as stpool,
            tc.tile_pool(name="small", bufs=1) as smpool,
            tc.tile_pool(name="vpool", bufs=2) as vpool,
            tc.tile_pool(name="psum", bufs=2, space="PSUM") as pspool,
            tc.tile_pool(name="dram", bufs=2, space="DRAM") as drpool,
        ):
            # ---- persistent SBUF residents ----
            xt = cpool.tile([128, NCH * 128], BF16, tag="xt")
            wr = cpool.tile([128, NCH * 512], BF16, tag="wr")
            # split big loads across DMA queues
            nsl = 8
            wsl = NCH * 512 // nsl
            for s in range(nsl):
                nc.sync.dma_start(
                    out=wr[:, s * wsl:(s + 1) * wsl], in_=wr_d[:, s * wsl:(s + 1) * wsl]
                )
            xsl = NCH * 128 // 4
            for s in range(4):
                nc.sync.dma_start(
                    out=xt[:, s * xsl:(s + 1) * xsl], in_=xt_d[:, s * xsl:(s + 1) * xsl]
                )
            b_state = cpool.tile([128, c_loc * J], F32, tag="bstate")
            ct = cpool.tile([128, c_loc * J], BF16, tag="ct")

            def uhat_chunk(g, ps):
                """u_hat for chunk g (4 capsules) -> psum [128, 2048] f32."""
                for q in range(4):
                    nc.tensor.matmul(
                        ps[:, 512 * q:512 * (q + 1)],
                        xt[32 * q:32 * (q + 1), 128 * g:128 * (g + 1)],
                        wr[32 * q:32 * (q + 1), 512 * g:512 * (g + 1)],
                        start=True,
                        stop=True,
                        tile_position=(32 * q, 0),
                    )

            def supergroup_stage(sg):
                """Recompute u_hat for supergroup sg (16 c) into bf16 staging."""
                ust = stpool.tile([128, 16 * 512], BF16, tag="ust")
                nch_here = min(4, NCH - 4 * sg)
                for ch in range(nch_here):
                    g = 4 * sg + ch
                    ps = pspool.tile([128, 2048], F32, tag="ps")
                    uhat_chunk(g, ps)
                    nc.scalar.copy(ust[:, 2048 * ch:2048 * (ch + 1)], ps[:, :])
                return ust

            def squash(s_glob, v_f, v_b):
                """v = squash(s) over j per (b, u); s_glob [128, (u, j)] f32."""
                sq = smpool.tile([128, JU], F32, tag="sq")
                nc.scalar.square(sq[:, :], s_glob[:, :])
                msq = smpool.tile([128, U], F32, tag="msq")
                nc.vector.tensor_reduce(
                    msq[:, :], sq[:, :].rearrange("p (u j) -> p u j", u=U, j=J),
                    axis=AX.X, op=ALU.add,
                )
                msqe = smpool.tile([128, U], F32, tag="msqe")
                nc.vector.tensor_scalar_add(msqe[:, :], msq[:, :], EPS)
                mag = smpool.tile([128, U], F32, tag="mag")
                nc.scalar.activation(mag[:, :], msqe[:, :], ACTF.Sqrt)
                magpe = smpool.tile([128, U], F32, tag="magpe")
                nc.vector.tensor_scalar_add(magpe[:, :], mag[:, :], EPS)
                onep = smpool.tile([128, U], F32, tag="onep")
                nc.vector.tensor_scalar_add(onep[:, :], msq[:, :], 1.0)
                den = smpool.tile([128, U], F32, tag="den")
                nc.vector.tensor_mul(den[:, :], onep[:, :], magpe[:, :])
                rec = smpool.tile([128, U], F32, tag="rec")
                nc.vector.reciprocal(rec[:, :], den[:, :])
                fac = smpool.tile([128, U], F32, tag="fac")
                nc.vector.tensor_mul(fac[:, :], msq[:, :], rec[:, :])
                fac_bc = fac[:, :].unsqueeze(2).broadcast_to([128, U, J])
                nc.vector.tensor_mul(
                    v_f[:, :].rearrange("p (u j) -> p u j", u=U, j=J),
                    s_glob[:, :].rearrange("p (u j) -> p u j", u=U, j=J),
                    fac_bc,
                )
                nc.vector.tensor_copy(v_b[:, :], v_f[:, :])

            def allreduce(s_loc):
                if not do_ar:
                    return s_loc
                bi = drpool.tile([128, JU], F32, tag="bi")
                bo = drpool.tile([128, JU], F32, tag="bo")
                nc.gpsimd.dma_start(out=bi[:, :], in_=s_loc[:, :])
                nc.gpsimd.collective_compute(
                    "AllReduce",
                    ALU.add,
                    replica_groups=[list(range(n_cores))],
                    ins=[bi.opt()],
                    outs=[bo.opt()],
                )
                s_glob = smpool.tile([128, JU], F32, tag="sglob")
                nc.gpsimd.dma_start(out=s_glob[:, :], in_=bo[:, :])
                return s_glob

            # ======== Phase 0: s0 = (1/J) * sum_c u_hat ========
            for _rep in range(reps):
              ps0 = pspool.tile([128, 2048], F32, tag="ps")
              for g in range(NCH):
                  nc.tensor.matmul(
                      ps0[:, :512],
                      xt[:, 128 * g:128 * (g + 1)],
                      wr[:, 512 * g:512 * (g + 1)],
                      start=(g == 0),
                      stop=(g == NCH - 1),
                  )
              s_loc = smpool.tile([128, JU], F32, tag="sloc")
              nc.scalar.mul(s_loc[:, :], ps0[:, :512], 1.0 / J)
              s_glob = allreduce(s_loc)
              v_f = vpool.tile([128, JU], F32, tag="vf")
              v_b = vpool.tile([128, JU], BF16, tag="vb")
              squash(s_glob, v_f, v_b)
              tap("s0", s_glob[:, :])
              tap("v0", v_f[:, :])

              # ======== routing iterations ========
              for t in range(1, n_iters):
                  # ---- du-phase: b_state += sum_u u_hat * v ----
                  for sg in range(NSG):
                      ncs = min(16, c_loc - 16 * sg)  # capsules in this sg
                      ust = supergroup_stage(sg)
                      if t == 1 and sg == 0:
                          tap("us0", ust[:, :])
                      tmp = stpool.tile([128, 16 * 512], BF16, tag="tmp", bufs=1)
                      v_bc = (
                          v_b[:, :].unsqueeze(1).broadcast_to([128, ncs, 512])
                      )
                      nc.vector.tensor_mul(
                          tmp[:, :ncs * 512].rearrange("p (c f) -> p c f", c=ncs),
                          ust[:, :ncs * 512].rearrange("p (c f) -> p c f", c=ncs),
                          v_bc,
                      )
                      red_in = tmp[:, :ncs * 512].rearrange(
                          "p (c u j) -> p c j u", c=ncs, u=U, j=J
                      )
                      bsl = b_state[:, 512 * sg:512 * sg + ncs * J].rearrange(
                          "p (c j) -> p c j", c=ncs
                      )
                      if t == 1:
                          nc.vector.tensor_reduce(bsl, red_in, axis=AX.X, op=ALU.add)
                      else:
                          du = smpool.tile([128, 512], F32, tag="du")
                          nc.vector.tensor_reduce(
                              du[:, :ncs * J].rearrange("p (c j) -> p c j", c=ncs),
                              red_in, axis=AX.X, op=ALU.add,
                          )
                          nc.vector.tensor_add(
                              b_state[:, 512 * sg:512 * sg + ncs * J],
                              b_state[:, 512 * sg:512 * sg + ncs * J],
                              du[:, :ncs * J],
                          )
                  tap("b1" if t == 1 else "b2", b_state[:, :])
                  # ---- softmax over j -> ct (bf16) ----
                  # shift logits by per-(b,c) max (softmax-invariant; prevents
                  # exp overflow). In-place: b_state keeps shifted logits.
                  mmax = smpool.tile([128, c_loc], F32, tag="mmax")
                  nc.vector.tensor_reduce(
                      mmax[:, :],
                      b_state[:, :].rearrange("p (c j) -> p c j", j=J),
                      axis=AX.X, op=ALU.max,
                  )
                  mmax_bc = mmax[:, :].unsqueeze(2).broadcast_to([128, c_loc, J])
                  nc.vector.tensor_sub(
                      b_state[:, :].rearrange("p (c j) -> p c j", j=J),
                      b_state[:, :].rearrange("p (c j) -> p c j", j=J),
                      mmax_bc,
                  )
                  ncols = c_loc * J
                  hcols = min(4096, ncols)
                  for h in range((ncols + hcols - 1) // hcols):
                      lo = h * hcols
                      w = min(hcols, ncols - lo)
                      ncs = w // J
                      expf = stpool.tile([128, 4096], F32, tag="tmp", bufs=1)
                      nc.scalar.activation(
                          expf[:, :w], b_state[:, lo:lo + w], ACTF.Exp
                      )
                      sums = smpool.tile([128, 128], F32, tag="sums")
                      nc.vector.tensor_reduce(
                          sums[:, :ncs],
                          expf[:, :w].rearrange("p (c j) -> p c j", j=J),
                          axis=AX.X, op=ALU.add,
                      )
                      rec = smpool.tile([128, 128], F32, tag="srec")
                      nc.vector.reciprocal(rec[:, :ncs], sums[:, :ncs])
                      rec_bc = (
                          rec[:, :ncs].unsqueeze(2).broadcast_to([128, ncs, J])
                      )
                      nc.vector.tensor_mul(
                          ct[:, lo:lo + w].rearrange("p (c j) -> p c j", j=J),
                          expf[:, :w].rearrange("p (c j) -> p c j", j=J),
                          rec_bc,
                      )
                  tap("ct1" if t == 1 else "ct2", ct[:, :])
                  # ---- s-phase: s = sum_c ct * u_hat ----
                  s_loc = smpool.tile([128, JU], F32, tag="sloc")
                  for sg in range(NSG):
                      ncs = min(16, c_loc - 16 * sg)
                      ust = supergroup_stage(sg)
                      tmp2 = stpool.tile([128, 16 * 512], BF16, tag="tmp", bufs=1)
                      ct_bc = (
                          ct[:, 512 * sg:512 * sg + ncs * J]
                          .rearrange("p (c j) -> p c j", c=ncs)
                          .unsqueeze(2)
                          .broadcast_to([128, ncs, U, J])
                      )
                      nc.vector.tensor_mul(
                          tmp2[:, :ncs * 512].rearrange(
                              "p (c u j) -> p c u j", c=ncs, u=U, j=J
                          ),
                          ust[:, :ncs * 512].rearrange(
                              "p (c u j) -> p c u j", c=ncs, u=U, j=J
                          ),
                          ct_bc,
                      )
                      red2 = tmp2[:, :ncs * 512].rearrange(
                          "p (c f) -> p f c", c=ncs
                      )
                      if sg == 0:
                          nc.vector.tensor_reduce(
                              s_loc[:, :], red2, axis=AX.X, op=ALU.add
                          )
                      else:
                          sp = smpool.tile([128, 512], F32, tag="du")
                          nc.vector.tensor_reduce(sp[:, :], red2, axis=AX.X, op=ALU.add)
                          nc.vector.tensor_add(s_loc[:, :], s_loc[:, :], sp[:, :])
                  if t == 1:
                      tap("s1", s_loc[:, :])
                  s_glob = allreduce(s_loc)
                  v_f = vpool.tile([128, JU], F32, tag="vf")
                  v_b = vpool.tile([128, JU], BF16, tag="vb")
                  squash(s_glob, v_f, v_b)

            nc.sync.dma_start(out=out_d[:, :], in_=v_f[:, :])

    nc.finalize()
    return nc


# ---------------- host-side layout prep ----------------

def prep_core_inputs(x, W0, c0, c_loc):
    """x [B, C, I] f32, W0 [C, J, I, U] f32 -> {'xt', 'wr'} bf16 arrays."""
    NCH = c_loc // 4
    xs = np.asarray(x[:, c0:c0 + c_loc, :], dtype=np.float32)   # [B, c_loc, I]
    Ws = np.asarray(W0[c0:c0 + c_loc], dtype=np.float32)        # [c_loc, J, I, U]
    # xt[32q+i, 128g+b] = x[b, 4g+q, i]
    xt = np.zeros((NCH, 4, 32, B), dtype=np.float32)
    xt[:, :, :I, :] = xs.transpose(1, 2, 0).reshape(NCH, 4, I, B)
    xt = xt.reshape(NCH, 128, B).transpose(1, 0, 2).reshape(128, NCH * B)
    # wr[32q+i, 512g + 32u + j] = W[4g+q, j, i, u]
    wr = np.zeros((NCH, 4, 32, U, J), dtype=np.float32)
    wr[:, :, :I, :, :] = Ws.transpose(0, 2, 3, 1).reshape(NCH, 4, I, U, J)
    wr = wr.reshape(NCH, 128, U * J).transpose(1, 0, 2).reshape(128, NCH * U * J)
    return {
        "xt": xt.astype(ml_dtypes.bfloat16),
        "wr": wr.astype(ml_dtypes.bfloat16),
    }


def postprocess(out_core):
    """[128, (u, j)] f32 -> [B, J, U]."""
    return np.asarray(out_core).reshape(B, U, J).transpose(0, 2, 1).copy()


_NC_CACHE = {}


def kernel(x, W):
    from concourse.bass_utils import run_bass_kernel_spmd

    n_cores = 8
    c_loc = C // n_cores
    key = (c_loc, n_cores)
    if key not in _NC_CACHE:
        _NC_CACHE[key] = build_nc(c_loc=c_loc, n_cores=n_cores)
    nc = _NC_CACHE[key]
    W0 = np.asarray(W[0], dtype=np.float32)
    in_maps = [
        prep_core_inputs(x, W0, i * c_loc, c_loc) for i in range(n_cores)
    ]
    res = run_bass_kernel_spmd(nc, in_maps, core_ids=list(range(n_cores)))
    return postprocess(res.results[0]["out"]).astype(np.float32)



# revision 22
# speedup vs baseline: 342.4161x; 342.4161x over previous
"""CapsuleLayer dynamic-routing kernel for one TRN2 chip (8 NeuronCores).

Self-contained graded entry point: kernel(x, W) -> [128, 32, 16] float32.

Sharding: capsules C=2048 are split across the 8 cores (c_loc=256 each);
B=128 rides the partition axis so the PE array is fully used. The three
per-iteration s-vectors are combined with a small AllReduce ([128,512] f32).

Per routing iteration the kernel makes ONE pass over the per-core u_hat
(recomputed on the PE in supergroups of 16 capsules), fusing the b-update
(reduce over u), the softmax over j, and the s-accumulation (reduce over c)
into the same pass. Reductions use log-tree tensor_tensor adds in bf16
(2x DVE mode) instead of tensor_reduce (1x mode).
"""

import sys

sys.path.insert(0, "/opt/trn_rl_repo")

import numpy as np
import ml_dtypes

import concourse.bass as bass
import concourse.bacc as bacc
import concourse.mybir as mybir
from concourse.tile import TileContext

BF16 = mybir.dt.bfloat16
F32 = mybir.dt.float32
AX = mybir.AxisListType
ALU = mybir.AluOpType
ACTF = mybir.ActivationFunctionType

B, C, I, J, U = 128, 2048, 16, 32, 16
JU = J * U  # 512
EPS = 1e-8
N_ITERS = 3


def build_nc(c_loc=256, n_cores=8, n_iters=N_ITERS, do_ar=True, reps=1,
             hw_loop=False, debug_taps=False):
    NCH = c_loc // 4          # chunks of 4 capsules
    NSG = (NCH + 3) // 4      # supergroups of 4 chunks (16 capsules)
    assert NCH % 4 == 0
    nc = bacc.Bacc(None, target_bir_lowering=False, debug=False)
    xt_d = nc.declare_dram_parameter("xt", [128, NCH * 128], BF16, isOutput=False)
    wr_d = nc.declare_dram_parameter("wr", [128, NCH * 512], BF16, isOutput=False)
    out_d = nc.declare_dram_parameter("out", [128, JU], F32, isOutput=True)
    taps = {}
    if debug_taps:
        for nm, cols, dt in [("s0", JU, F32), ("v0", JU, F32),
                             ("b1", c_loc * J, F32), ("s1", JU, F32),
                             ("v1", JU, F32), ("b2", c_loc * J, F32),
                             ("us0", 16 * 512, BF16), ("ct0", 512, BF16)]:
            taps[nm] = nc.declare_dram_parameter("tap_" + nm, [128, cols], dt,
                                                 isOutput=True)

    def tap(nm, ap):
        if debug_taps:
            nc.sync.dma_start(out=taps[nm][:, :], in_=ap)

    with TileContext(nc) as tc:
        with (
            tc.tile_pool(name="const", bufs=1) as cpool,
            tc.tile_pool(name="stage", bufs=2) as stpool,
            tc.tile_pool(name="work", bufs=1) as wpool,
            tc.tile_pool(name="small", bufs=2) as smpool,
            tc.tile_pool(name="ser", bufs=1) as sepool,
            tc.tile_pool(name="vsm", bufs=2) as vpool,
            tc.tile_pool(name="psum", bufs=2, space="PSUM") as pspool,
            tc.tile_pool(name="dram", bufs=2, space="DRAM") as drpool,
        ):
            # ---- persistent SBUF residents ----
            xt = cpool.tile([128, NCH * 128], BF16, tag="xt")
            wr = cpool.tile([128, NCH * 512], BF16, tag="wr")
            nsl = 8
            wsl = NCH * 512 // nsl
            for s in range(nsl):
                nc.sync.dma_start(
                    out=wr[:, s * wsl:(s + 1) * wsl], in_=wr_d[:, s * wsl:(s + 1) * wsl]
                )
            xsl = NCH * 128 // 4
            for s in range(4):
                nc.sync.dma_start(
                    out=xt[:, s * xsl:(s + 1) * xsl], in_=xt_d[:, s * xsl:(s + 1) * xsl]
                )
            b_state = cpool.tile([128, c_loc * J], F32, tag="bstate")

            def stage_sg(sg):
                """u_hat for supergroup sg (16 caps) -> bf16 [128, 16*512].

                Columns within a capsule are (u major, j minor)."""
                ust = stpool.tile([128, 16 * 512], BF16, tag="ust")
                for ch in range(4):
                    g = 4 * sg + ch
                    ps = pspool.tile([128, 2048], F32, tag="ps")
                    for q in range(4):
                        nc.tensor.matmul(
                            ps[:, 512 * q:512 * (q + 1)],
                            xt[32 * q:32 * (q + 1), 128 * g:128 * (g + 1)],
                            wr[32 * q:32 * (q + 1), 512 * g:512 * (g + 1)],
                            start=True,
                            stop=True,
                            tile_position=(32 * q, 0),
                        )
                    nc.scalar.copy(ust[:, 2048 * ch:2048 * (ch + 1)], ps[:, :])
                return ust

            def halve_blk(src_ap, nblk, seg, out_ap):
                """out[p, c, seg] = src[p, c, 0, seg] + src[p, c, 1, seg]."""
                v = src_ap.rearrange("p (c h f) -> p c h f", c=nblk, h=2, f=seg)
                ov = out_ap.rearrange("p (c f) -> p c f", c=nblk, f=seg)
                nc.vector.tensor_add(ov, v[:, :, 0, :], v[:, :, 1, :])

            def squash(s_glob, v_f, v_b):
                """v = squash(s) over j per (b, u); s_glob [128, (u, j)] f32."""
                sq = sepool.tile([128, JU], F32, tag="sq")
                nc.scalar.square(sq[:, :], s_glob[:, :])
                msq = sepool.tile([128, U], F32, tag="msq")
                nc.vector.tensor_reduce(
                    msq[:, :], sq[:, :].rearrange("p (u j) -> p u j", u=U, j=J),
                    axis=AX.X, op=ALU.add,
                )
                msqe = sepool.tile([128, U], F32, tag="msqe")
                nc.vector.tensor_scalar_add(msqe[:, :], msq[:, :], EPS)
                mag = sepool.tile([128, U], F32, tag="mag")
                nc.scalar.activation(mag[:, :], msqe[:, :], ACTF.Sqrt)
                magpe = sepool.tile([128, U], F32, tag="magpe")
                nc.vector.tensor_scalar_add(magpe[:, :], mag[:, :], EPS)
                onep = sepool.tile([128, U], F32, tag="onep")
                nc.vector.tensor_scalar_add(onep[:, :], msq[:, :], 1.0)
                den = sepool.tile([128, U], F32, tag="den")
                nc.vector.tensor_mul(den[:, :], onep[:, :], magpe[:, :])
                rec = sepool.tile([128, U], F32, tag="rec")
                nc.vector.reciprocal(rec[:, :], den[:, :])
                fac = sepool.tile([128, U], F32, tag="fac")
                nc.vector.tensor_mul(fac[:, :], msq[:, :], rec[:, :])
                fac_bc = fac[:, :].unsqueeze(2).broadcast_to([128, U, J])
                nc.vector.tensor_mul(
                    v_f[:, :].rearrange("p (u j) -> p u j", u=U, j=J),
                    s_glob[:, :].rearrange("p (u j) -> p u j", u=U, j=J),
                    fac_bc,
                )
                nc.vector.tensor_copy(v_b[:, :], v_f[:, :])

            def allreduce(s_loc):
                if not do_ar:
                    return s_loc
                bi = drpool.tile([128, JU], F32, tag="bi")
                bo = drpool.tile([128, JU], F32, tag="bo")
                nc.gpsimd.dma_start(out=bi[:, :], in_=s_loc[:, :])
                nc.gpsimd.collective_compute(
                    "AllReduce",
                    ALU.add,
                    replica_groups=[list(range(n_cores))],
                    ins=[bi.opt()],
                    outs=[bo.opt()],
                )
                s_glob = sepool.tile([128, JU], F32, tag="sglob")
                nc.gpsimd.dma_start(out=s_glob[:, :], in_=bo[:, :])
                return s_glob

            def rep_body():
                # ======== Phase 0: s0 = (1/J) * sum_c u_hat ========
                ps0_full = pspool.tile([128, 2048], F32, tag="ps")
                ps0 = ps0_full[:, :512]
                for g in range(NCH):
                    nc.tensor.matmul(
                        ps0[:, :],
                        xt[:, 128 * g:128 * (g + 1)],
                        wr[:, 512 * g:512 * (g + 1)],
                        start=(g == 0),
                        stop=(g == NCH - 1),
                    )
                s_loc = sepool.tile([128, JU], F32, tag="sloc")
                nc.scalar.mul(s_loc[:, :], ps0[:, :], 1.0 / J)
                s_glob = allreduce(s_loc)
                v_f = vpool.tile([128, JU], F32, tag="vf")
                v_b = vpool.tile([128, JU], BF16, tag="vb")
                squash(s_glob, v_f, v_b)
                tap("s0", s_glob[:, :])
                tap("v0", v_f[:, :])

                # ======== routing iterations (fused, sw-pipelined) ========
                # Per iteration, one pass over supergroups. The s-phase of
                # supergroup k-1 is emitted after the du-phase of k so the
                # ACT Exp latency hides under DVE work.
                def du_phase(t, sg, ust):
                    wk = wpool.tile([128, 16 * 512], BF16, tag="wk")
                    aux = wpool.tile([128, 4096], BF16, tag="aux")
                    v_bc = v_b[:, :].unsqueeze(1).broadcast_to([128, 16, 512])
                    nc.vector.tensor_mul(
                        wk[:, :].rearrange("p (c f) -> p c f", c=16, f=512),
                        ust[:, :].rearrange("p (c f) -> p c f", c=16, f=512),
                        v_bc,
                    )
                    halve_blk(wk[:, :], 16, 256, aux[:, :])
                    halve_blk(aux[:, :], 16, 128, wk[:, :2048])
                    halve_blk(wk[:, :2048], 16, 64, wk[:, 2048:3072])
                    bsl = b_state[:, 512 * sg:512 * (sg + 1)]
                    if t == 1:
                        halve_blk(wk[:, 2048:3072], 16, 32, bsl)
                    else:
                        du = sepool.tile([128, 512], F32, tag="du")
                        halve_blk(wk[:, 2048:3072], 16, 32, du[:, :])
                        nc.vector.tensor_add(bsl, bsl, du[:, :])
                    # softmax over j per capsule. Logits are shifted by the
                    # per-partition max over this supergroup via the ACT
                    # Exp bias (softmax-invariant); underflowed capsules
                    # are guarded by the +1e-30 on Z.
                    bslv = bsl.rearrange("p (c j) -> p c j", c=16, j=J)
                    negmx = smpool.tile([128, 1], F32, tag="negmx")
                    nc.vector.tensor_reduce(negmx[:, :], bslv, axis=AX.XY,
                                            op=ALU.max, negate=True)
                    expf = smpool.tile([128, 512], BF16, tag="expf")
                    nc.scalar.activation(expf[:, :], bsl, ACTF.Exp,
                                         bias=negmx[:, :])
                    zs = sepool.tile([128, 16], F32, tag="zs")
                    nc.vector.tensor_reduce(
                        zs[:, :],
                        expf[:, :].rearrange("p (c j) -> p c j", c=16, j=J),
                        axis=AX.X, op=ALU.add,
                    )
                    nc.vector.tensor_scalar_add(zs[:, :], zs[:, :], 1e-30)
                    rec = sepool.tile([128, 16], F32, tag="rec16")
                    nc.vector.reciprocal(rec[:, :], zs[:, :])
                    ct = smpool.tile([128, 512], BF16, tag="ct")
                    rec_bc = rec[:, :].unsqueeze(2).broadcast_to([128, 16, J])
                    nc.vector.tensor_mul(
                        ct[:, :].rearrange("p (c j) -> p c j", c=16, j=J),
                        expf[:, :].rearrange("p (c j) -> p c j", c=16, j=J),
                        rec_bc,
                    )
                    return ct

                def s_phase(sg, ust, ct, s_loc):
                    wk = wpool.tile([128, 16 * 512], BF16, tag="wk")
                    aux = wpool.tile([128, 4096], BF16, tag="aux")
                    ct_bc = (
                        ct[:, :].rearrange("p (c j) -> p c j", c=16, j=J)
                        .unsqueeze(2)
                        .broadcast_to([128, 16, U, J])
                    )
                    nc.vector.tensor_mul(
                        wk[:, :].rearrange("p (c u j) -> p c u j", c=16, u=U, j=J),
                        ust[:, :].rearrange("p (c u j) -> p c u j", c=16, u=U, j=J),
                        ct_bc,
                    )
                    halve_blk(wk[:, :], 1, 4096, aux[:, :])
                    halve_blk(aux[:, :], 1, 2048, wk[:, :2048])
                    halve_blk(wk[:, :2048], 1, 1024, wk[:, 2048:3072])
                    if sg == 0:
                        halve_blk(wk[:, 2048:3072], 1, 512, s_loc[:, :])
                    else:
                        sp = sepool.tile([128, 512], F32, tag="sp")
                        halve_blk(wk[:, 2048:3072], 1, 512, sp[:, :])
                        nc.vector.tensor_add(s_loc[:, :], s_loc[:, :], sp[:, :])

                s_final = None
                for t in range(1, n_iters):
                    s_loc = vpool.tile([128, JU], F32, tag="sloc")
                    ust_prev = None
                    ct_prev = None
                    for k in range(NSG + 1):
                        if k < NSG:
                            ust = stage_sg(k)
                            if t == 1 and k == 0:
                                tap("us0", ust[:, :])
                            ct = du_phase(t, k, ust)
                            if t == 1 and k == 0:
                                tap("ct0", ct[:, :])
                        if k >= 1:
                            s_phase(k - 1, ust_prev, ct_prev, s_loc)
                        if k < NSG:
                            ust_prev, ct_prev = ust, ct
                    tap("b1" if t == 1 else "b2", b_state[:, :])
                    if t == 1:
                        tap("s1", s_loc[:, :])
                    if t < n_iters - 1:
                        s_glob = allreduce(s_loc)
                        v_f = vpool.tile([128, JU], F32, tag="vf")
                        v_b = vpool.tile([128, JU], BF16, tag="vb")
                        squash(s_glob, v_f, v_b)
                        if t == 1:
                            tap("v1", v_f[:, :])
                    else:
                        s_final = s_loc
                return s_final

            if hw_loop:
                with tc.For_i(0, reps):
                    s_final = rep_body()
            else:
                for _rep in range(reps):
                    s_final = rep_body()

            nc.sync.dma_start(out=out_d[:, :], in_=s_final[:, :])

    nc.finalize()
    return nc


# ---------------- host-side layout prep ----------------

def prep_core_inputs(x, W0, c0, c_loc):
    """x [B, C, I] f32, W0 [C, J, I, U] f32 -> {'xt', 'wr'} bf16 arrays."""
    NCH = c_loc // 4
    xs = np.asarray(x[:, c0:c0 + c_loc, :], dtype=np.float32)   # [B, c_loc, I]
    Ws = np.asarray(W0[c0:c0 + c_loc], dtype=np.float32)        # [c_loc, J, I, U]
    # xt[32q+i, 128g+b] = x[b, 4g+q, i]
    xt = np.zeros((NCH, 4, 32, B), dtype=np.float32)
    xt[:, :, :I, :] = xs.transpose(1, 2, 0).reshape(NCH, 4, I, B)
    xt = xt.reshape(NCH, 128, B).transpose(1, 0, 2).reshape(128, NCH * B)
    # wr[32q+i, 512g + 32u + j] = W[4g+q, j, i, u]
    wr = np.zeros((NCH, 4, 32, U, J), dtype=np.float32)
    wr[:, :, :I, :, :] = Ws.transpose(0, 2, 3, 1).reshape(NCH, 4, I, U, J)
    wr = wr.reshape(NCH, 128, U * J).transpose(1, 0, 2).reshape(128, NCH * U * J)
    return {
        "xt": xt.astype(ml_dtypes.bfloat16),
        "wr": wr.astype(ml_dtypes.bfloat16),
    }


def postprocess(out_cores):
    """Per-core s2 partials [128, (u, j)] f32 -> final v [B, J, U].

    The last AllReduce + squash run host-side as part of unsharding:
    s2 = sum over cores, v = squash(s2) over j per (b, u).
    """
    s = np.zeros((128, JU), np.float64)
    for oc in out_cores:
        s += np.asarray(oc, np.float64)
    s3 = s.reshape(B, U, J)
    msq = (s3 * s3).sum(axis=2, keepdims=True)
    mag = np.sqrt(msq + EPS)
    v = msq / (1.0 + msq) * (s3 / (mag + EPS))
    return v.transpose(0, 2, 1).astype(np.float32).copy()


_NC_CACHE = {}


def kernel(x, W):
    from concourse.bass_utils import run_bass_kernel_spmd

    n_cores = 8
    c_loc = C // n_cores
    key = (c_loc, n_cores)
    if key not in _NC_CACHE:
        _NC_CACHE[key] = build_nc(c_loc=c_loc, n_cores=n_cores)
    nc = _NC_CACHE[key]
    W0 = np.asarray(W[0], dtype=np.float32)
    in_maps = [
        prep_core_inputs(x, W0, i * c_loc, c_loc) for i in range(n_cores)
    ]
    res = run_bass_kernel_spmd(nc, in_maps, core_ids=list(range(n_cores)))
    return postprocess([r["out"] for r in res.results])


# revision 23
# speedup vs baseline: 1710.4620x; 4.9953x over previous
"""CapsuleLayer dynamic-routing kernel for one TRN2 chip (8 NeuronCores).

Self-contained graded entry point: kernel(x, W) -> [128, 32, 16] float32.

Sharding: capsules C=2048 are split across the 8 cores (c_loc=256 each);
B=128 rides the partition axis so the PE array is fully used. The three
per-iteration s-vectors are combined with a small AllReduce ([128,512] f32).

Per routing iteration the kernel makes ONE pass over the per-core u_hat
(recomputed on the PE in supergroups of 16 capsules), fusing the b-update
(reduce over u), the softmax over j, and the s-accumulation (reduce over c)
into the same pass. Reductions use log-tree tensor_tensor adds in bf16
(2x DVE mode) instead of tensor_reduce (1x mode).
"""

import sys

sys.path.insert(0, "/opt/trn_rl_repo")

import numpy as np
import ml_dtypes

import concourse.bass as bass
import concourse.bacc as bacc
import concourse.mybir as mybir
from concourse.tile import TileContext

BF16 = mybir.dt.bfloat16
F32 = mybir.dt.float32
AX = mybir.AxisListType
ALU = mybir.AluOpType
ACTF = mybir.ActivationFunctionType

B, C, I, J, U = 128, 2048, 16, 32, 16
JU = J * U  # 512
EPS = 1e-8
N_ITERS = 3


def build_nc(c_loc=256, n_cores=8, n_iters=N_ITERS, do_ar=True, reps=1,
             hw_loop=False, debug_taps=False):
    NCH = c_loc // 4          # chunks of 4 capsules
    NSG = (NCH + 3) // 4      # supergroups of 4 chunks (16 capsules)
    assert NCH % 4 == 0
    nc = bacc.Bacc(None, target_bir_lowering=False, debug=False)
    xt_d = nc.declare_dram_parameter("xt", [128, NCH * 128], BF16, isOutput=False)
    wr_d = nc.declare_dram_parameter("wr", [128, NCH * 512], BF16, isOutput=False)
    out_d = nc.declare_dram_parameter("out", [128, JU], F32, isOutput=True)
    taps = {}
    if debug_taps:
        for nm, cols, dt in [("s0", JU, F32), ("v0", JU, F32),
                             ("b1", c_loc * J, F32), ("s1", JU, F32),
                             ("v1", JU, F32), ("b2", c_loc * J, F32),
                             ("us0", 16 * 512, BF16), ("ct0", 512, BF16)]:
            taps[nm] = nc.declare_dram_parameter("tap_" + nm, [128, cols], dt,
                                                 isOutput=True)

    def tap(nm, ap):
        if debug_taps:
            nc.sync.dma_start(out=taps[nm][:, :], in_=ap)

    with TileContext(nc) as tc:
        with (
            tc.tile_pool(name="const", bufs=1) as cpool,
            tc.tile_pool(name="stage", bufs=3) as stpool,
            tc.tile_pool(name="work", bufs=1) as wpool,
            tc.tile_pool(name="small", bufs=2) as smpool,
            tc.tile_pool(name="ser", bufs=1) as sepool,
            tc.tile_pool(name="vsm", bufs=2) as vpool,
            tc.tile_pool(name="psum", bufs=2, space="PSUM") as pspool,
            tc.tile_pool(name="dram", bufs=2, space="DRAM") as drpool,
        ):
            # ---- persistent SBUF residents ----
            xt = cpool.tile([128, NCH * 128], BF16, tag="xt")
            wr = cpool.tile([128, NCH * 512], BF16, tag="wr")
            nsl = 8
            wsl = NCH * 512 // nsl
            for s in range(nsl):
                nc.sync.dma_start(
                    out=wr[:, s * wsl:(s + 1) * wsl], in_=wr_d[:, s * wsl:(s + 1) * wsl]
                )
            xsl = NCH * 128 // 4
            for s in range(4):
                nc.sync.dma_start(
                    out=xt[:, s * xsl:(s + 1) * xsl], in_=xt_d[:, s * xsl:(s + 1) * xsl]
                )
            b_state = cpool.tile([128, c_loc * J], F32, tag="bstate")

            def stage_sg(sg):
                """u_hat for supergroup sg (16 caps) -> bf16 [128, 16*512].

                Columns within a capsule are (u major, j minor)."""
                ust = stpool.tile([128, 16 * 512], BF16, tag="ust")
                for ch in range(4):
                    g = 4 * sg + ch
                    ps = pspool.tile([128, 2048], F32, tag="ps")
                    for q in range(4):
                        nc.tensor.matmul(
                            ps[:, 512 * q:512 * (q + 1)],
                            xt[32 * q:32 * (q + 1), 128 * g:128 * (g + 1)],
                            wr[32 * q:32 * (q + 1), 512 * g:512 * (g + 1)],
                            start=True,
                            stop=True,
                            tile_position=(32 * q, 0),
                        )
                    nc.scalar.copy(ust[:, 2048 * ch:2048 * (ch + 1)], ps[:, :])
                return ust

            def halve_blk(src_ap, nblk, seg, out_ap):
                """out[p, c, seg] = src[p, c, 0, seg] + src[p, c, 1, seg]."""
                v = src_ap.rearrange("p (c h f) -> p c h f", c=nblk, h=2, f=seg)
                ov = out_ap.rearrange("p (c f) -> p c f", c=nblk, f=seg)
                nc.vector.tensor_add(ov, v[:, :, 0, :], v[:, :, 1, :])

            def squash(s_glob, v_f, v_b):
                """v = squash(s) over j per (b, u); s_glob [128, (u, j)] f32."""
                sq = sepool.tile([128, JU], F32, tag="sq")
                nc.scalar.square(sq[:, :], s_glob[:, :])
                msq = sepool.tile([128, U], F32, tag="msq")
                nc.vector.tensor_reduce(
                    msq[:, :], sq[:, :].rearrange("p (u j) -> p u j", u=U, j=J),
                    axis=AX.X, op=ALU.add,
                )
                msqe = sepool.tile([128, U], F32, tag="msqe")
                nc.vector.tensor_scalar_add(msqe[:, :], msq[:, :], EPS)
                mag = sepool.tile([128, U], F32, tag="mag")
                nc.scalar.activation(mag[:, :], msqe[:, :], ACTF.Sqrt)
                magpe = sepool.tile([128, U], F32, tag="magpe")
                nc.vector.tensor_scalar_add(magpe[:, :], mag[:, :], EPS)
                onep = sepool.tile([128, U], F32, tag="onep")
                nc.vector.tensor_scalar_add(onep[:, :], msq[:, :], 1.0)
                den = sepool.tile([128, U], F32, tag="den")
                nc.vector.tensor_mul(den[:, :], onep[:, :], magpe[:, :])
                rec = sepool.tile([128, U], F32, tag="rec")
                nc.vector.reciprocal(rec[:, :], den[:, :])
                fac = sepool.tile([128, U], F32, tag="fac")
                nc.vector.tensor_mul(fac[:, :], msq[:, :], rec[:, :])
                fac_bc = fac[:, :].unsqueeze(2).broadcast_to([128, U, J])
                nc.vector.tensor_mul(
                    v_f[:, :].rearrange("p (u j) -> p u j", u=U, j=J),
                    s_glob[:, :].rearrange("p (u j) -> p u j", u=U, j=J),
                    fac_bc,
                )
                nc.vector.tensor_copy(v_b[:, :], v_f[:, :])

            def allreduce(s_loc):
                if not do_ar:
                    return s_loc
                bi = drpool.tile([128, JU], F32, tag="bi")
                bo = drpool.tile([128, JU], F32, tag="bo")
                nc.gpsimd.dma_start(out=bi[:, :], in_=s_loc[:, :])
                nc.gpsimd.collective_compute(
                    "AllReduce",
                    ALU.add,
                    replica_groups=[list(range(n_cores))],
                    ins=[bi.opt()],
                    outs=[bo.opt()],
                )
                s_glob = sepool.tile([128, JU], F32, tag="sglob")
                nc.gpsimd.dma_start(out=s_glob[:, :], in_=bo[:, :])
                return s_glob

            def rep_body():
                # ======== Phase 0: s0 = (1/J) * sum_c u_hat ========
                ps0_full = pspool.tile([128, 2048], F32, tag="ps")
                ps0 = ps0_full[:, :512]
                for g in range(NCH):
                    nc.tensor.matmul(
                        ps0[:, :],
                        xt[:, 128 * g:128 * (g + 1)],
                        wr[:, 512 * g:512 * (g + 1)],
                        start=(g == 0),
                        stop=(g == NCH - 1),
                    )
                s_loc = sepool.tile([128, JU], F32, tag="sloc")
                nc.scalar.mul(s_loc[:, :], ps0[:, :], 1.0 / J)
                s_glob = allreduce(s_loc)
                v_f = vpool.tile([128, JU], F32, tag="vf")
                v_b = vpool.tile([128, JU], BF16, tag="vb")
                squash(s_glob, v_f, v_b)
                tap("s0", s_glob[:, :])
                tap("v0", v_f[:, :])

                # ======== routing iterations (fused, sw-pipelined) ========
                # Per iteration, one pass over supergroups. The s-phase of
                # supergroup k-1 is emitted after the du-phase of k so the
                # ACT Exp latency hides under DVE work.
                def du_phase(t, sg, ust):
                    wk = wpool.tile([128, 16 * 512], BF16, tag="wk")
                    aux = wpool.tile([128, 4096], BF16, tag="aux")
                    v_bc = v_b[:, :].unsqueeze(1).broadcast_to([128, 16, 512])
                    nc.vector.tensor_mul(
                        wk[:, :].rearrange("p (c f) -> p c f", c=16, f=512),
                        ust[:, :].rearrange("p (c f) -> p c f", c=16, f=512),
                        v_bc,
                    )
                    halve_blk(wk[:, :], 16, 256, aux[:, :])
                    halve_blk(aux[:, :], 16, 128, wk[:, :2048])
                    halve_blk(wk[:, :2048], 16, 64, wk[:, 2048:3072])
                    bsl = b_state[:, 512 * sg:512 * (sg + 1)]
                    if t == 1:
                        halve_blk(wk[:, 2048:3072], 16, 32, bsl)
                    else:
                        du = sepool.tile([128, 512], F32, tag="du")
                        halve_blk(wk[:, 2048:3072], 16, 32, du[:, :])
                        nc.vector.tensor_add(bsl, bsl, du[:, :])
                    # softmax over j per capsule. Logits are shifted by the
                    # per-partition max over this supergroup via the ACT
                    # Exp bias (softmax-invariant); underflowed capsules
                    # are guarded by the +1e-30 on Z.
                    bslv = bsl.rearrange("p (c j) -> p c j", c=16, j=J)
                    negmx = smpool.tile([128, 1], F32, tag="negmx")
                    nc.vector.tensor_reduce(negmx[:, :], bslv, axis=AX.XY,
                                            op=ALU.max, negate=True)
                    expf = smpool.tile([128, 512], BF16, tag="expf")
                    nc.scalar.activation(expf[:, :], bsl, ACTF.Exp,
                                         bias=negmx[:, :])
                    zs = sepool.tile([128, 16], F32, tag="zs")
                    nc.vector.tensor_reduce(
                        zs[:, :],
                        expf[:, :].rearrange("p (c j) -> p c j", c=16, j=J),
                        axis=AX.X, op=ALU.add,
                    )
                    nc.vector.tensor_scalar_add(zs[:, :], zs[:, :], 1e-30)
                    rec = sepool.tile([128, 16], F32, tag="rec16")
                    nc.vector.reciprocal(rec[:, :], zs[:, :])
                    ct = smpool.tile([128, 512], BF16, tag="ct")
                    rec_bc = rec[:, :].unsqueeze(2).broadcast_to([128, 16, J])
                    nc.vector.tensor_mul(
                        ct[:, :].rearrange("p (c j) -> p c j", c=16, j=J),
                        expf[:, :].rearrange("p (c j) -> p c j", c=16, j=J),
                        rec_bc,
                    )
                    return ct

                def s_phase(sg, ust, ct, s_loc):
                    wk = wpool.tile([128, 16 * 512], BF16, tag="wk")
                    aux = wpool.tile([128, 4096], BF16, tag="aux")
                    ct_bc = (
                        ct[:, :].rearrange("p (c j) -> p c j", c=16, j=J)
                        .unsqueeze(2)
                        .broadcast_to([128, 16, U, J])
                    )
                    nc.vector.tensor_mul(
                        wk[:, :].rearrange("p (c u j) -> p c u j", c=16, u=U, j=J),
                        ust[:, :].rearrange("p (c u j) -> p c u j", c=16, u=U, j=J),
                        ct_bc,
                    )
                    halve_blk(wk[:, :], 1, 4096, aux[:, :])
                    halve_blk(aux[:, :], 1, 2048, wk[:, :2048])
                    halve_blk(wk[:, :2048], 1, 1024, wk[:, 2048:3072])
                    if sg == 0:
                        halve_blk(wk[:, 2048:3072], 1, 512, s_loc[:, :])
                    else:
                        sp = sepool.tile([128, 512], F32, tag="du")
                        halve_blk(wk[:, 2048:3072], 1, 512, sp[:, :])
                        nc.vector.tensor_add(s_loc[:, :], s_loc[:, :], sp[:, :])

                s_final = None
                for t in range(1, n_iters):
                    s_loc = vpool.tile([128, JU], F32, tag="sloc")
                    ust_prev = None
                    ct_prev = None
                    for k in range(NSG + 1):
                        if k < NSG:
                            ust = stage_sg(k)
                            if t == 1 and k == 0:
                                tap("us0", ust[:, :])
                            ct = du_phase(t, k, ust)
                            if t == 1 and k == 0:
                                tap("ct0", ct[:, :])
                        if k >= 1:
                            s_phase(k - 1, ust_prev, ct_prev, s_loc)
                        if k < NSG:
                            ust_prev, ct_prev = ust, ct
                    tap("b1" if t == 1 else "b2", b_state[:, :])
                    if t == 1:
                        tap("s1", s_loc[:, :])
                    if t < n_iters - 1:
                        s_glob = allreduce(s_loc)
                        v_f = vpool.tile([128, JU], F32, tag="vf")
                        v_b = vpool.tile([128, JU], BF16, tag="vb")
                        squash(s_glob, v_f, v_b)
                        if t == 1:
                            tap("v1", v_f[:, :])
                    else:
                        s_final = s_loc
                return s_final

            if hw_loop:
                with tc.For_i(0, reps):
                    s_final = rep_body()
            else:
                for _rep in range(reps):
                    s_final = rep_body()

            nc.sync.dma_start(out=out_d[:, :], in_=s_final[:, :])

    nc.finalize()
    return nc


# ---------------- host-side layout prep ----------------

def prep_core_inputs(x, W0, c0, c_loc):
    """x [B, C, I] f32, W0 [C, J, I, U] f32 -> {'xt', 'wr'} bf16 arrays."""
    NCH = c_loc // 4
    xs = np.asarray(x[:, c0:c0 + c_loc, :], dtype=np.float32)   # [B, c_loc, I]
    Ws = np.asarray(W0[c0:c0 + c_loc], dtype=np.float32)        # [c_loc, J, I, U]
    # xt[32q+i, 128g+b] = x[b, 4g+q, i]
    xt = np.zeros((NCH, 4, 32, B), dtype=np.float32)
    xt[:, :, :I, :] = xs.transpose(1, 2, 0).reshape(NCH, 4, I, B)
    xt = xt.reshape(NCH, 128, B).transpose(1, 0, 2).reshape(128, NCH * B)
    # wr[32q+i, 512g + 32u + j] = W[4g+q, j, i, u]
    wr = np.zeros((NCH, 4, 32, U, J), dtype=np.float32)
    wr[:, :, :I, :, :] = Ws.transpose(0, 2, 3, 1).reshape(NCH, 4, I, U, J)
    wr = wr.reshape(NCH, 128, U * J).transpose(1, 0, 2).reshape(128, NCH * U * J)
    return {
        "xt": xt.astype(ml_dtypes.bfloat16),
        "wr": wr.astype(ml_dtypes.bfloat16),
    }


def postprocess(out_cores):
    """Per-core s2 partials [128, (u, j)] f32 -> final v [B, J, U].

    The last AllReduce + squash run host-side as part of unsharding:
    s2 = sum over cores, v = squash(s2) over j per (b, u).
    """
    s = np.zeros((128, JU), np.float64)
    for oc in out_cores:
        s += np.asarray(oc, np.float64)
    s3 = s.reshape(B, U, J)
    msq = (s3 * s3).sum(axis=2, keepdims=True)
    mag = np.sqrt(msq + EPS)
    v = msq / (1.0 + msq) * (s3 / (mag + EPS))
    return v.transpose(0, 2, 1).astype(np.float32).copy()


_NC_CACHE = {}


def kernel(x, W):
    from concourse.bass_utils import run_bass_kernel_spmd

    n_cores = 8
    c_loc = C // n_cores
    key = (c_loc, n_cores)
    if key not in _NC_CACHE:
        _NC_CACHE[key] = build_nc(c_loc=c_loc, n_cores=n_cores)
    nc = _NC_CACHE[key]
    W0 = np.asarray(W[0], dtype=np.float32)
    in_maps = [
        prep_core_inputs(x, W0, i * c_loc, c_loc) for i in range(n_cores)
    ]
    res = run_bass_kernel_spmd(nc, in_maps, core_ids=list(range(n_cores)))
    return postprocess([r["out"] for r in res.results])


# revision 27
# speedup vs baseline: 1751.6274x; 1.0241x over previous
"""CapsuleLayer dynamic-routing kernel for one TRN2 chip (8 NeuronCores).

Self-contained graded entry point: kernel(x, W) -> [128, 32, 16] float32.

Sharding: capsules C=2048 are split across the 8 cores (c_loc=256 each);
B=128 rides the partition axis so the PE array is fully used. The three
per-iteration s-vectors are combined with a small AllReduce ([128,512] f32).

Per routing iteration the kernel makes ONE pass over the per-core u_hat
(recomputed on the PE in supergroups of 16 capsules), fusing the b-update
(reduce over u), the softmax over j, and the s-accumulation (reduce over c)
into the same pass. Reductions use log-tree tensor_tensor adds in bf16
(2x DVE mode) instead of tensor_reduce (1x mode).
"""

import sys

sys.path.insert(0, "/opt/trn_rl_repo")

import numpy as np
import ml_dtypes

import concourse.bass as bass
import concourse.bacc as bacc
import concourse.mybir as mybir
from concourse.tile import TileContext

BF16 = mybir.dt.bfloat16
F32 = mybir.dt.float32
AX = mybir.AxisListType
ALU = mybir.AluOpType
ACTF = mybir.ActivationFunctionType

B, C, I, J, U = 128, 2048, 16, 32, 16
JU = J * U  # 512
EPS = 1e-8
N_ITERS = 3


def build_nc(c_loc=256, n_cores=8, n_iters=N_ITERS, do_ar=True, reps=1,
             hw_loop=False, debug_taps=False):
    NCH = c_loc // 4          # chunks of 4 capsules
    NSG = (NCH + 3) // 4      # supergroups of 4 chunks (16 capsules)
    assert NCH % 4 == 0
    nc = bacc.Bacc(None, target_bir_lowering=False, debug=False)
    xt_d = nc.declare_dram_parameter("xt", [128, NCH * 128], BF16, isOutput=False)
    wr_d = nc.declare_dram_parameter("wr", [128, NCH * 512], BF16, isOutput=False)
    out_d = nc.declare_dram_parameter("out", [128, JU], F32, isOutput=True)
    taps = {}
    if debug_taps:
        for nm, cols, dt in [("s0", JU, F32), ("v0", JU, F32),
                             ("b1", c_loc * J, F32), ("s1", JU, F32),
                             ("v1", JU, F32), ("b2", c_loc * J, F32),
                             ("us0", 16 * 512, BF16), ("ct0", 512, BF16)]:
            taps[nm] = nc.declare_dram_parameter("tap_" + nm, [128, cols], dt,
                                                 isOutput=True)

    def tap(nm, ap):
        if debug_taps:
            nc.sync.dma_start(out=taps[nm][:, :], in_=ap)

    with TileContext(nc) as tc:
        with (
            tc.tile_pool(name="const", bufs=1) as cpool,
            tc.tile_pool(name="stage", bufs=3) as stpool,
            tc.tile_pool(name="work", bufs=1) as wpool,
            tc.tile_pool(name="small", bufs=2) as smpool,
            tc.tile_pool(name="ser", bufs=1) as sepool,
            tc.tile_pool(name="vsm", bufs=2) as vpool,
            tc.tile_pool(name="psum", bufs=2, space="PSUM") as pspool,
            tc.tile_pool(name="dram", bufs=2, space="DRAM") as drpool,
        ):
            # ---- persistent SBUF residents ----
            xt = cpool.tile([128, NCH * 128], BF16, tag="xt")
            wr = cpool.tile([128, NCH * 512], BF16, tag="wr")
            nsl = 8
            wsl = NCH * 512 // nsl
            for s in range(nsl):
                nc.sync.dma_start(
                    out=wr[:, s * wsl:(s + 1) * wsl], in_=wr_d[:, s * wsl:(s + 1) * wsl]
                )
            xsl = NCH * 128 // 4
            for s in range(4):
                nc.sync.dma_start(
                    out=xt[:, s * xsl:(s + 1) * xsl], in_=xt_d[:, s * xsl:(s + 1) * xsl]
                )
            b_state = cpool.tile([128, c_loc * J], F32, tag="bstate")

            def stage_sg(sg):
                """u_hat for supergroup sg (16 caps) -> bf16 [128, 16*512].

                Columns within a capsule are (u major, j minor)."""
                ust = stpool.tile([128, 16 * 512], BF16, tag="ust")
                for ch in range(4):
                    g = 4 * sg + ch
                    ps = pspool.tile([128, 2048], F32, tag="ps")
                    for q in range(4):
                        nc.tensor.matmul(
                            ps[:, 512 * q:512 * (q + 1)],
                            xt[32 * q:32 * (q + 1), 128 * g:128 * (g + 1)],
                            wr[32 * q:32 * (q + 1), 512 * g:512 * (g + 1)],
                            start=True,
                            stop=True,
                            tile_position=(32 * q, 0),
                        )
                    nc.scalar.copy(ust[:, 2048 * ch:2048 * (ch + 1)], ps[:, :])
                return ust

            def halve_blk(src_ap, nblk, seg, out_ap):
                """out[p, c, seg] = src[p, c, 0, seg] + src[p, c, 1, seg]."""
                v = src_ap.rearrange("p (c h f) -> p c h f", c=nblk, h=2, f=seg)
                ov = out_ap.rearrange("p (c f) -> p c f", c=nblk, f=seg)
                nc.vector.tensor_add(ov, v[:, :, 0, :], v[:, :, 1, :])

            def squash(s_glob, v_f, v_b):
                """v = squash(s) over j per (b, u); s_glob [128, (u, j)] f32."""
                sq = sepool.tile([128, JU], F32, tag="du")
                nc.scalar.square(sq[:, :], s_glob[:, :])
                msq = sepool.tile([128, U], F32, tag="msq")
                nc.vector.tensor_reduce(
                    msq[:, :], sq[:, :].rearrange("p (u j) -> p u j", u=U, j=J),
                    axis=AX.X, op=ALU.add,
                )
                msqe = sepool.tile([128, U], F32, tag="msqe")
                nc.vector.tensor_scalar_add(msqe[:, :], msq[:, :], EPS)
                mag = sepool.tile([128, U], F32, tag="mag")
                nc.scalar.activation(mag[:, :], msqe[:, :], ACTF.Sqrt)
                magpe = sepool.tile([128, U], F32, tag="magpe")
                nc.vector.tensor_scalar_add(magpe[:, :], mag[:, :], EPS)
                onep = sepool.tile([128, U], F32, tag="onep")
                nc.vector.tensor_scalar_add(onep[:, :], msq[:, :], 1.0)
                den = sepool.tile([128, U], F32, tag="den")
                nc.vector.tensor_mul(den[:, :], onep[:, :], magpe[:, :])
                rec = sepool.tile([128, U], F32, tag="rec")
                nc.vector.reciprocal(rec[:, :], den[:, :])
                fac = sepool.tile([128, U], F32, tag="fac")
                nc.vector.tensor_mul(fac[:, :], msq[:, :], rec[:, :])
                fac_bc = fac[:, :].unsqueeze(2).broadcast_to([128, U, J])
                nc.vector.tensor_mul(
                    v_f[:, :].rearrange("p (u j) -> p u j", u=U, j=J),
                    s_glob[:, :].rearrange("p (u j) -> p u j", u=U, j=J),
                    fac_bc,
                )
                nc.vector.tensor_copy(v_b[:, :], v_f[:, :])

            def allreduce(s_loc):
                if not do_ar:
                    return s_loc
                bi = drpool.tile([128, JU], F32, tag="bi")
                bo = drpool.tile([128, JU], F32, tag="bo")
                nc.gpsimd.dma_start(out=bi[:, :], in_=s_loc[:, :])
                nc.gpsimd.collective_compute(
                    "AllReduce",
                    ALU.add,
                    replica_groups=[list(range(n_cores))],
                    ins=[bi.opt()],
                    outs=[bo.opt()],
                )
                s_glob = sepool.tile([128, JU], F32, tag="sglob")
                nc.gpsimd.dma_start(out=s_glob[:, :], in_=bo[:, :])
                return s_glob

            def rep_body():
                # ======== Phase 0: s0 = (1/J) * sum_c u_hat ========
                ps0_full = pspool.tile([128, 2048], F32, tag="ps")
                ps0 = ps0_full[:, :512]
                for g in range(NCH):
                    nc.tensor.matmul(
                        ps0[:, :],
                        xt[:, 128 * g:128 * (g + 1)],
                        wr[:, 512 * g:512 * (g + 1)],
                        start=(g == 0),
                        stop=(g == NCH - 1),
                    )
                s_loc = sepool.tile([128, JU], F32, tag="sloc")
                nc.scalar.mul(s_loc[:, :], ps0[:, :], 1.0 / J)
                s_glob = allreduce(s_loc)
                v_f = vpool.tile([128, JU], F32, tag="vf")
                v_b = vpool.tile([128, JU], BF16, tag="vb")
                squash(s_glob, v_f, v_b)
                tap("s0", s_glob[:, :])
                tap("v0", v_f[:, :])

                # ======== routing iterations (fused, sw-pipelined) ========
                # Per iteration, one pass over supergroups. The s-phase of
                # supergroup k-1 is emitted after the du-phase of k so the
                # ACT Exp latency hides under DVE work.
                def du_phase(t, sg, ust):
                    wk = wpool.tile([128, 16 * 512], BF16, tag="wk")
                    aux = wpool.tile([128, 4096], BF16, tag="aux")
                    v_bc = v_b[:, :].unsqueeze(1).broadcast_to([128, 16, 512])
                    nc.vector.tensor_mul(
                        wk[:, :].rearrange("p (c f) -> p c f", c=16, f=512),
                        ust[:, :].rearrange("p (c f) -> p c f", c=16, f=512),
                        v_bc,
                    )
                    halve_blk(wk[:, :], 16, 256, aux[:, :])
                    halve_blk(aux[:, :], 16, 128, wk[:, :2048])
                    halve_blk(wk[:, :2048], 16, 64, wk[:, 2048:3072])
                    bsl = b_state[:, 512 * sg:512 * (sg + 1)]
                    if t == 1:
                        halve_blk(wk[:, 2048:3072], 16, 32, bsl)
                    else:
                        du = sepool.tile([128, 512], F32, tag="du")
                        halve_blk(wk[:, 2048:3072], 16, 32, du[:, :])
                        nc.vector.tensor_add(bsl, bsl, du[:, :])
                    # softmax over j per capsule. Logits are shifted by the
                    # per-partition max over this supergroup via the ACT
                    # Exp bias (softmax-invariant); underflowed capsules
                    # are guarded by the +1e-30 on Z.
                    bslv = bsl.rearrange("p (c j) -> p c j", c=16, j=J)
                    negmx = smpool.tile([128, 1], F32, tag="negmx")
                    nc.vector.tensor_reduce(negmx[:, :], bslv, axis=AX.XY,
                                            op=ALU.max, negate=True)
                    expf = sepool.tile([128, 512], BF16, tag="expf")
                    nc.scalar.activation(expf[:, :], bsl, ACTF.Exp,
                                         bias=negmx[:, :])
                    zs = sepool.tile([128, 16], F32, tag="zs")
                    nc.vector.tensor_reduce(
                        zs[:, :],
                        expf[:, :].rearrange("p (c j) -> p c j", c=16, j=J),
                        axis=AX.X, op=ALU.add,
                    )
                    nc.vector.tensor_scalar_add(zs[:, :], zs[:, :], 1e-30)
                    rec = sepool.tile([128, 16], F32, tag="rec16")
                    nc.vector.reciprocal(rec[:, :], zs[:, :])
                    ct = smpool.tile([128, 512], BF16, tag="ct")
                    rec_bc = rec[:, :].unsqueeze(2).broadcast_to([128, 16, J])
                    nc.vector.tensor_mul(
                        ct[:, :].rearrange("p (c j) -> p c j", c=16, j=J),
                        expf[:, :].rearrange("p (c j) -> p c j", c=16, j=J),
                        rec_bc,
                    )
                    return ct

                def s_phase(sg, ust, ct, s_acc):
                    wk = wpool.tile([128, 16 * 512], BF16, tag="wk")
                    aux = wpool.tile([128, 4096], BF16, tag="aux")
                    ct_bc = (
                        ct[:, :].rearrange("p (c j) -> p c j", c=16, j=J)
                        .unsqueeze(2)
                        .broadcast_to([128, 16, U, J])
                    )
                    nc.vector.tensor_mul(
                        wk[:, :].rearrange("p (c u j) -> p c u j", c=16, u=U, j=J),
                        ust[:, :].rearrange("p (c u j) -> p c u j", c=16, u=U, j=J),
                        ct_bc,
                    )
                    halve_blk(wk[:, :], 1, 4096, aux[:, :])
                    if sg == 0:
                        halve_blk(aux[:, :], 1, 2048, s_acc[:, :])
                    else:
                        halve_blk(aux[:, :], 1, 2048, wk[:, :2048])
                        nc.vector.tensor_add(s_acc[:, :], s_acc[:, :],
                                             wk[:, :2048])

                s_final = None
                for t in range(1, n_iters):
                    # bf16 partial-sum accumulator at the 2048-col tree level;
                    # folded to f32 s_loc once per iteration.
                    s_acc = sepool.tile([128, 2048], BF16, tag="sacc")
                    ust_prev = None
                    ct_prev = None
                    for k in range(NSG + 1):
                        if k < NSG:
                            ust = stage_sg(k)
                            if t == 1 and k == 0:
                                tap("us0", ust[:, :])
                            ct = du_phase(t, k, ust)
                            if t == 1 and k == 0:
                                tap("ct0", ct[:, :])
                        if k >= 1:
                            s_phase(k - 1, ust_prev, ct_prev, s_acc)
                        if k < NSG:
                            ust_prev, ct_prev = ust, ct
                    fold = wpool.tile([128, 4096], BF16, tag="aux")
                    halve_blk(s_acc[:, :], 1, 1024, fold[:, :1024])
                    s_loc = vpool.tile([128, JU], F32, tag="sloc")
                    halve_blk(fold[:, :1024], 1, 512, s_loc[:, :])
                    tap("b1" if t == 1 else "b2", b_state[:, :])
                    if t == 1:
                        tap("s1", s_loc[:, :])
                    if t < n_iters - 1:
                        s_glob = allreduce(s_loc)
                        v_f = vpool.tile([128, JU], F32, tag="vf")
                        v_b = vpool.tile([128, JU], BF16, tag="vb")
                        squash(s_glob, v_f, v_b)
                        if t == 1:
                            tap("v1", v_f[:, :])
                    else:
                        s_final = s_loc
                return s_final

            if hw_loop:
                with tc.For_i(0, reps):
                    s_final = rep_body()
            else:
                for _rep in range(reps):
                    s_final = rep_body()

            nc.sync.dma_start(out=out_d[:, :], in_=s_final[:, :])

    nc.finalize()
    return nc


# ---------------- host-side layout prep ----------------

def prep_core_inputs(x, W0, c0, c_loc):
    """x [B, C, I] f32, W0 [C, J, I, U] f32 -> {'xt', 'wr'} bf16 arrays."""
    NCH = c_loc // 4
    xs = np.asarray(x[:, c0:c0 + c_loc, :], dtype=np.float32)   # [B, c_loc, I]
    Ws = np.asarray(W0[c0:c0 + c_loc], dtype=np.float32)        # [c_loc, J, I, U]
    # xt[32q+i, 128g+b] = x[b, 4g+q, i]
    xt = np.zeros((NCH, 4, 32, B), dtype=np.float32)
    xt[:, :, :I, :] = xs.transpose(1, 2, 0).reshape(NCH, 4, I, B)
    xt = xt.reshape(NCH, 128, B).transpose(1, 0, 2).reshape(128, NCH * B)
    # wr[32q+i, 512g + 32u + j] = W[4g+q, j, i, u]
    wr = np.zeros((NCH, 4, 32, U, J), dtype=np.float32)
    wr[:, :, :I, :, :] = Ws.transpose(0, 2, 3, 1).reshape(NCH, 4, I, U, J)
    wr = wr.reshape(NCH, 128, U * J).transpose(1, 0, 2).reshape(128, NCH * U * J)
    return {
        "xt": xt.astype(ml_dtypes.bfloat16),
        "wr": wr.astype(ml_dtypes.bfloat16),
    }


def postprocess(out_cores):
    """Per-core s2 partials [128, (u, j)] f32 -> final v [B, J, U].

    The last AllReduce + squash run host-side as part of unsharding:
    s2 = sum over cores, v = squash(s2) over j per (b, u).
    """
    s = np.zeros((128, JU), np.float64)
    for oc in out_cores:
        s += np.asarray(oc, np.float64)
    s3 = s.reshape(B, U, J)
    msq = (s3 * s3).sum(axis=2, keepdims=True)
    mag = np.sqrt(msq + EPS)
    v = msq / (1.0 + msq) * (s3 / (mag + EPS))
    return v.transpose(0, 2, 1).astype(np.float32).copy()


_NC_CACHE = {}


def kernel(x, W):
    from concourse.bass_utils import run_bass_kernel_spmd

    n_cores = 8
    c_loc = C // n_cores
    key = (c_loc, n_cores)
    if key not in _NC_CACHE:
        _NC_CACHE[key] = build_nc(c_loc=c_loc, n_cores=n_cores)
    nc = _NC_CACHE[key]
    W0 = np.asarray(W[0], dtype=np.float32)
    in_maps = [
        prep_core_inputs(x, W0, i * c_loc, c_loc) for i in range(n_cores)
    ]
    res = run_bass_kernel_spmd(nc, in_maps, core_ids=list(range(n_cores)))
    return postprocess([r["out"] for r in res.results])
